# revision 5
# baseline (speedup 1.0000x reference)
"""Sliding-window attention TRN2 kernel (nn_Attention_89764816486949).

Sharding: 8 cores = 4 head-groups x 2 batches. Core c handles batch (c % 2)
and heads [4*(c//2) .. 4*(c//2)+3]. Each core computes its partial output
projection outT [D, T] = sum_{its heads} w_out[h].T @ encoded[h].T; the host
sums the 4 partials per batch and transposes.

All matmuls run as float32r (TF32-like, ~1.4e-4 rel err, full PE rate).
"""
import sys
sys.path.insert(0, '/opt/trn_rl_repo')

import numpy as np

B, T, D, N, H = 2, 2048, 2048, 16, 128
WINDOW = 1024
SOFT_CAP = 50.0
MAX_WAVELENGTH = 10000

HPC = 4            # heads per core
TB = 512           # token block (free dim of most matmuls)
NTB = T // TB      # 4
NK = D // 128      # 16 contraction tiles
NJ = T // 128      # 16 key tiles per batch
NCORES = 8

_compiled = {}


def _build_nc():
    import concourse.bacc as bacc
    import concourse.mybir as mybir
    from concourse import tile

    F32 = mybir.dt.float32
    F32R = mybir.dt.float32r
    AF = mybir.ActivationFunctionType
    OP = mybir.AluOpType

    nc = bacc.Bacc(None, target_bir_lowering=False, debug=False)

    xt_d = nc.dram_tensor("xt", [D, T], F32R, kind="ExternalInput").ap()
    wq_d = nc.dram_tensor("wq", [HPC, D, H], F32R, kind="ExternalInput").ap()
    wk_d = nc.dram_tensor("wk", [HPC, D, H], F32R, kind="ExternalInput").ap()
    wv01_d = nc.dram_tensor("wv01", [D, 2 * H], F32R, kind="ExternalInput").ap()
    wv23_d = nc.dram_tensor("wv23", [D, 2 * H], F32R, kind="ExternalInput").ap()
    wo_d = nc.dram_tensor("wo", [HPC, H, D], F32R, kind="ExternalInput").ap()
    cos_d = nc.dram_tensor("ropecos", [H, T], F32, kind="ExternalInput").ap()
    sin_d = nc.dram_tensor("ropesin", [H, T], F32, kind="ExternalInput").ap()
    maskc_d = nc.dram_tensor("maskc", [128, 896], F32, kind="ExternalInput").ap()
    maskw_d = nc.dram_tensor("maskw", [128, 896], F32, kind="ExternalInput").ap()
    ones_d = nc.dram_tensor("ones", [128, 1], F32R, kind="ExternalInput").ap()
    onesrow_d = nc.dram_tensor("onesrow", [1, 128], F32R, kind="ExternalInput").ap()
    outt_d = nc.dram_tensor("outt", [D, T], F32, kind="ExternalOutput").ap()

    with tile.TileContext(nc) as tc:
        with tc.tile_pool(name="outer", bufs=1) as outer:
            cos_sb = outer.tile([H, T], F32, tag="cos")
            nc.sync.dma_start(out=cos_sb[:, :], in_=cos_d[:, :])
            sin_sb = outer.tile([H, T], F32, tag="sin")
            nc.sync.dma_start(out=sin_sb[:, :], in_=sin_d[:, :])
            maskc_sb = outer.tile([128, 896], F32, tag="maskc")
            nc.sync.dma_start(out=maskc_sb[:, :], in_=maskc_d[:, :])
            maskw_sb = outer.tile([128, 896], F32, tag="maskw")
            nc.sync.dma_start(out=maskw_sb[:, :], in_=maskw_d[:, :])
            ones_sb = outer.tile([128, 1], F32R, tag="ones")
            nc.sync.dma_start(out=ones_sb[:, :], in_=ones_d[:, :])
            onesrow_sb = outer.tile([1, 128], F32R, tag="onesrow")
            nc.sync.dma_start(out=onesrow_sb[:, :], in_=onesrow_d[:, :])
            enc_sb = [outer.tile([H, T], F32R, tag=f"enc{h}", name=f"enc{h}")
                      for h in range(HPC)]

            for p in range(2):  # head pairs (2p, 2p+1)
                with tc.tile_pool(name=f"pass{p}", bufs=1) as pp:
                    # resident weights for this pair
                    wq_sb, wk_sb, qT, kT, v_sb = [], [], [], [], []
                    for hh in range(2):
                        h = 2 * p + hh
                        wt = pp.tile([128, NK * H], F32R, tag=f"wq{hh}")
                        nc.sync.dma_start(
                            out=wt[:, :].rearrange("p (k j) -> p k j", j=H),
                            in_=wq_d[h].rearrange("(k p) j -> p k j", p=128))
                        wq_sb.append(wt)
                        wt = pp.tile([128, NK * H], F32R, tag=f"wk{hh}")
                        nc.sync.dma_start(
                            out=wt[:, :].rearrange("p (k j) -> p k j", j=H),
                            in_=wk_d[h].rearrange("(k p) j -> p k j", p=128))
                        wk_sb.append(wt)
                        qT.append(pp.tile([H, T], F32R, tag=f"qT{hh}", name=f"qT{hh}"))
                        kT.append(pp.tile([H, T], F32R, tag=f"kT{hh}", name=f"kT{hh}"))
                        v_sb.append(pp.tile([128, T], F32R, tag=f"v{hh}", name=f"v{hh}"))
                    wv_sb = pp.tile([128, NK * 2 * H], F32R, tag="wv")
                    nc.sync.dma_start(
                        out=wv_sb[:, :].rearrange("p (k j) -> p k j", j=2 * H),
                        in_=(wv01_d if p == 0 else wv23_d).rearrange(
                            "(k p) j -> p k j", p=128))

                    # ---- projection ----
                    with tc.tile_pool(name=f"proj_ps{p}", bufs=1, space="PSUM") as ps_pool:
                        for tb in range(NTB):
                            psq = [ps_pool.tile([128, TB], F32, tag=f"pq{hh}",
                                                name=f"pq{hh}") for hh in range(2)]
                            psk = [ps_pool.tile([128, TB], F32, tag=f"pk{hh}",
                                                name=f"pk{hh}") for hh in range(2)]
                            psv = [ps_pool.tile([128, 256], F32, tag=f"pv{m}",
                                                name=f"pv{m}") for m in range(4)]
                            for k in range(NK):
                                xt = pp.tile([128, TB], F32R, tag="xt", bufs=6)
                                nc.sync.dma_start(
                                    out=xt[:, :],
                                    in_=xt_d[k * 128:(k + 1) * 128,
                                             tb * TB:(tb + 1) * TB])
                                st = (k == 0)
                                sp = (k == NK - 1)
                                for hh in range(2):
                                    nc.tensor.matmul(
                                        psq[hh][:, :],
                                        wq_sb[hh][:, k * H:(k + 1) * H],
                                        xt[:, :], start=st, stop=sp)
                                    nc.tensor.matmul(
                                        psk[hh][:, :],
                                        wk_sb[hh][:, k * H:(k + 1) * H],
                                        xt[:, :], start=st, stop=sp)
                                for m in range(4):
                                    nc.tensor.matmul(
                                        psv[m][:, :],
                                        xt[:, m * 128:(m + 1) * 128],
                                        wv_sb[:, k * 256:(k + 1) * 256],
                                        start=st, stop=sp)
                            # evict v: psv[a] cols [half*256 + hh*128] -> v_sb[hh]
                            for m in range(4):
                                for hh in range(2):
                                    nc.vector.tensor_copy(
                                        v_sb[hh][:, (tb * 4 + m) * 128:
                                                 (tb * 4 + m + 1) * 128],
                                        psv[m][:, hh * 128:(hh + 1) * 128])
                            # rope eviction for q and k
                            cosb = cos_sb[:, tb * TB:(tb + 1) * TB]
                            sinb = sin_sb[:, tb * TB:(tb + 1) * TB]
                            for hh in range(2):
                                for ps, dst in ((psq[hh], qT[hh]),
                                                (psk[hh], kT[hh])):
                                    dslice = dst[:, tb * TB:(tb + 1) * TB]
                                    raw = pp.tile([128, TB], F32, tag="raw", bufs=2)
                                    nc.scalar.activation(raw[:, :], ps[:, :],
                                                         AF.Copy)
                                    rot = pp.tile([128, TB], F32, tag="rot", bufs=2)
                                    nc.sync.dma_start(out=rot[0:64, :],
                                                      in_=raw[64:128, :])
                                    nc.sync.dma_start(out=rot[64:128, :],
                                                      in_=raw[0:64, :])
                                    t1 = pp.tile([128, TB], F32, tag="t1", bufs=2)
                                    nc.vector.tensor_tensor(
                                        out=t1[:, :], in0=rot[:, :], in1=sinb,
                                        op=OP.mult)
                                    nc.vector.tensor_tensor(
                                        out=dslice, in0=ps[:, :], in1=cosb,
                                        op=OP.mult)
                                    nc.vector.tensor_tensor(
                                        out=dslice, in0=dslice.bitcast(F32),
                                        in1=t1[:, :], op=OP.add)

                    # ---- attention for this head pair ----
                    with tc.tile_pool(name=f"attn_ps{p}", bufs=1, space="PSUM") as aps:
                        for hh in range(2):
                            h = 2 * p + hh
                            for g in range(NTB):
                                t0 = g * TB
                                jmin = max(0, t0 - (WINDOW - 1)) // 128
                                jmax = (t0 + TB - 1) // 128
                                pts = {}
                                for j in range(jmin, jmax + 1):
                                    stp = aps.tile([128, TB], F32, tag="st", bufs=3)
                                    nc.tensor.matmul(
                                        stp[:, :],
                                        kT[hh][:, j * 128:(j + 1) * 128],
                                        qT[hh][:, t0:t0 + TB],
                                        start=True, stop=True)
                                    pt = pp.tile([128, TB], F32R,
                                                 tag=f"pt{(j - jmin) % 4}")
                                    nc.scalar.activation(pt[:, :], stp[:, :],
                                                         AF.Tanh, scale=1.0 / SOFT_CAP)
                                    nc.scalar.activation(pt[:, :], pt[:, :],
                                                         AF.Exp, scale=SOFT_CAP)
                                    r = j - 4 * g
                                    m = 4 * g - j
                                    if 0 <= r <= 3:  # causal diagonal tiles
                                        off = 384 - 128 * r
                                        nc.vector.tensor_tensor(
                                            out=pt[:, :], in0=pt[:, :],
                                            in1=maskc_sb[:, off:off + TB],
                                            op=OP.mult)
                                    elif 5 <= m <= 8:  # window lower edge
                                        off = 128 * m - 640
                                        nc.vector.tensor_tensor(
                                            out=pt[:, :], in0=pt[:, :],
                                            in1=maskw_sb[:, off:off + TB],
                                            op=OP.mult)
                                    pts[j] = pt
                                sums = aps.tile([1, TB], F32, tag="sums", bufs=2)
                                encp = aps.tile([H, TB], F32, tag="encp", bufs=2)
                                for j in range(jmin, jmax + 1):
                                    nc.tensor.matmul(
                                        sums[:, :], ones_sb[:, :], pts[j][:, :],
                                        start=(j == jmin), stop=(j == jmax))
                                for j in range(jmin, jmax + 1):
                                    nc.tensor.matmul(
                                        encp[:, :],
                                        v_sb[hh][:, j * 128:(j + 1) * 128],
                                        pts[j][:, :],
                                        start=(j == jmin), stop=(j == jmax))
                                recip = pp.tile([1, TB], F32R, tag="recip", bufs=2)
                                with nc.allow_low_precision(reason="f32r softmax denom"):
                                    nc.vector.reciprocal(recip[:, :], sums[:, :])
                                rcb_ps = aps.tile([128, TB], F32, tag="rcb", bufs=1)
                                nc.tensor.matmul(rcb_ps[:, :], onesrow_sb[:, :],
                                                 recip[:, :], start=True, stop=True)
                                rcb = pp.tile([128, TB], F32, tag="rcbs", bufs=2)
                                nc.scalar.activation(rcb[:, :], rcb_ps[:, :],
                                                     AF.Copy)
                                nc.vector.tensor_tensor(
                                    out=enc_sb[h][:, t0:t0 + TB],
                                    in0=encp[:, :], in1=rcb[:, :], op=OP.mult)

            # ---- output projection: outT[d,t] = sum_h wo[h].T @ encT[h] ----
            with tc.tile_pool(name="oproj", bufs=1) as op_pool, \
                 tc.tile_pool(name="oproj_ps", bufs=1, space="PSUM") as ops:
                wo_sb = []
                for h in range(HPC):
                    wt = op_pool.tile([H, D], F32R, tag=f"wo{h}")
                    nc.sync.dma_start(out=wt[:, :], in_=wo_d[h])
                    wo_sb.append(wt)
                for d in range(D // 128):
                    for tb in range(NTB):
                        po = ops.tile([128, TB], F32, tag="po", bufs=4)
                        for h in range(HPC):
                            nc.tensor.matmul(
                                po[:, :],
                                wo_sb[h][:, d * 128:(d + 1) * 128],
                                enc_sb[h][:, tb * TB:(tb + 1) * TB],
                                start=(h == 0), stop=(h == HPC - 1))
                        osb = op_pool.tile([128, TB], F32, tag="osb", bufs=4)
                        nc.scalar.activation(osb[:, :], po[:, :], AF.Copy)
                        nc.sync.dma_start(
                            out=outt_d[d * 128:(d + 1) * 128,
                                       tb * TB:(tb + 1) * TB],
                            in_=osb[:, :])

    nc.compile()
    return nc


def _host_inputs(x, w_qkv, w_out, segment_pos):
    """Build the 8 per-core input maps."""
    scale = np.float32(H ** -0.5)
    in_maps = []
    # rope tables per batch (mirror the reference's fp32 arithmetic)
    fraction = (2.0 * np.arange(H // 2, dtype=np.float32) / np.float32(H)).astype(np.float32)
    timescale = np.power(np.float32(MAX_WAVELENGTH), fraction).astype(np.float32)
    tabs = []
    for b in range(B):
        ang = (segment_pos[b][:, None].astype(np.float32) / timescale[None, :])
        ang = ang.astype(np.float32)          # [T, 64]
        c = np.cos(ang).astype(np.float32).T  # [64, T]
        s = np.sin(ang).astype(np.float32).T
        cos_full = np.ascontiguousarray(np.concatenate([c, c], axis=0))
        sgn_sin = np.ascontiguousarray(np.concatenate([-s, s], axis=0))
        tabs.append((cos_full, sgn_sin))

    ds = np.arange(128)[:, None]
    u = np.arange(896)[None, :]
    maskc = (u - 384 >= ds).astype(np.float32)
    maskw = (u <= ds + 383).astype(np.float32)
    ones = np.ones((128, 1), np.float32)
    onesrow = np.ones((1, 128), np.float32)

    xts = [np.ascontiguousarray(x[b].T) for b in range(B)]

    for c in range(NCORES):
        b = c % 2
        hg = c // 2
        hs = hg * HPC
        wq = np.ascontiguousarray(w_qkv[0, hs:hs + HPC] * scale)
        wk = np.ascontiguousarray(w_qkv[1, hs:hs + HPC])
        wv01 = np.ascontiguousarray(
            np.concatenate([w_qkv[2, hs], w_qkv[2, hs + 1]], axis=1))
        wv23 = np.ascontiguousarray(
            np.concatenate([w_qkv[2, hs + 2], w_qkv[2, hs + 3]], axis=1))
        wo = np.ascontiguousarray(w_out[hs:hs + HPC])
        in_maps.append({
            "xt": xts[b], "wq": wq, "wk": wk, "wv01": wv01, "wv23": wv23,
            "wo": wo, "ropecos": tabs[b][0], "ropesin": tabs[b][1],
            "maskc": maskc, "maskw": maskw, "ones": ones, "onesrow": onesrow,
        })
    return in_maps


def kernel(x, w_qkv, w_out, segment_pos, attn_mask, _trace=False):
    from concourse.bass_utils import run_bass_kernel_spmd

    x = np.asarray(x, dtype=np.float32)
    w_qkv = np.asarray(w_qkv, dtype=np.float32)
    w_out = np.asarray(w_out, dtype=np.float32)
    segment_pos = np.asarray(segment_pos)

    if "nc" not in _compiled:
        _compiled["nc"] = _build_nc()
    nc = _compiled["nc"]

    in_maps = _host_inputs(x, w_qkv, w_out, segment_pos)
    r = run_bass_kernel_spmd(nc, in_maps, core_ids=list(range(NCORES)),
                             trace=_trace)
    _compiled["last_results"] = r

    out = np.zeros((B, T, D), np.float32)
    for b in range(B):
        acc = np.zeros((D, T), np.float64)
        for c in range(b, NCORES, 2):
            acc += r.results[c]["outt"]
        out[b] = acc.T.astype(np.float32)
    return out
